# revision 1
# baseline (speedup 1.0000x reference)
"""Trainium2 Bass kernel for AttentionPatcher (GQA attention block, S=2048).

Sharding: 8-way tensor parallel over KV head groups. Core c owns KV head c
and query heads 4c..4c+3: it computes its Q/K/V projections, RoPE, causal
attention, and a full partial o_proj (wo column shard); a ReduceScatter(add)
over the 8 cores then leaves core c with rows [512c, 512c+512) of the final
output, which the host concatenates.

All matmuls run as float32r (full-rate fp32-reduced) on the PE.
"""
import os
import sys

import numpy as np

if os.path.isdir("/opt/trn_rl_repo") and "/opt/trn_rl_repo" not in sys.path:
    sys.path.insert(0, "/opt/trn_rl_repo")

import concourse.bacc as bacc
import concourse.mybir as mybir
import concourse.tile as tile
from concourse.bass_utils import run_bass_kernel_spmd
from concourse.masks import make_identity

F32 = mybir.dt.float32
F32R = mybir.dt.float32r
ActF = mybir.ActivationFunctionType
Alu = mybir.AluOpType

H, KV, D, S = 32, 8, 128, 2048
HID = H * D
NCORES = 8
G = H // KV          # query heads per core
ST = 512             # s-tile size
NST = S // ST        # 4 s-tiles
KO = HID // 128      # 32 contraction subtiles
MO = HID // 128      # 32 output row tiles
INV_SQRT_D = 1.0 / float(np.sqrt(D))


def build_nc(with_collective=True):
    nc = bacc.Bacc("TRN2", target_bir_lowering=False, debug=False)

    x = nc.dram_tensor("x", [KO, 128, S], F32R, kind="ExternalInput")
    wq = nc.dram_tensor("wq", [KO, 128, G * 128], F32R, kind="ExternalInput")
    wk = nc.dram_tensor("wk", [128, KO, 128], F32R, kind="ExternalInput")
    wv = nc.dram_tensor("wv", [128, KO, 128], F32R, kind="ExternalInput")
    wo = nc.dram_tensor("wo", [MO, 128, G, 128], F32R, kind="ExternalInput")
    bq = nc.dram_tensor("bq", [128, G], F32, kind="ExternalInput")
    bk = nc.dram_tensor("bk", [128, 1], F32, kind="ExternalInput")
    bv = nc.dram_tensor("bv", [128, 1], F32, kind="ExternalInput")
    cos = nc.dram_tensor("cos", [128, S], F32, kind="ExternalInput")
    sin = nc.dram_tensor("sin", [128, S], F32, kind="ExternalInput")
    rot = nc.dram_tensor("rot", [128, 128], F32R, kind="ExternalInput")
    yout = nc.dram_tensor("y", [G, 128, S], F32, kind="ExternalOutput")

    with tile.TileContext(nc) as tc:
        with (
            tc.tile_pool(name="const", bufs=1) as const,
            tc.tile_pool(name="sb", bufs=3) as sb,
            tc.tile_pool(name="ps", bufs=1, space="PSUM") as ps,
            tc.tile_pool(name="dram", bufs=1, space="DRAM") as dram,
        ):
            # ---- resident constants ----
            # constants go through the scalar/gpsimd DMA queues (chunked)
            # so the sync queue starts streaming x/wq immediately
            wk_sb = const.tile([128, KO, 128], F32R)
            wv_sb = const.tile([128, KO, 128], F32R)
            for c8 in range(8):
                ksl = slice(c8 * (KO // 8), (c8 + 1) * (KO // 8))
                nc.scalar.dma_start(wk_sb[:, ksl, :], wk[:, ksl, :])
                nc.scalar.dma_start(wv_sb[:, ksl, :], wv[:, ksl, :])
            cos_sb = const.tile([128, S], F32)
            sin_sb = const.tile([128, S], F32)
            nc.scalar.dma_start(cos_sb[:], cos[:, :])
            nc.gpsimd.dma_start(sin_sb[:], sin[:, :])
            rot_sb = const.tile([128, 128], F32R)
            nc.scalar.dma_start(rot_sb[:], rot[:, :])
            bq_sb = const.tile([128, G], F32)
            bk_sb = const.tile([128, 1], F32)
            bv_sb = const.tile([128, 1], F32)
            nc.scalar.dma_start(bq_sb[:], bq[:, :])
            nc.scalar.dma_start(bk_sb[:], bk[:, :])
            nc.scalar.dma_start(bv_sb[:], bv[:, :])
            ones_f = const.tile([128, 128], F32)
            nc.any.memset(ones_f[:], 1.0)
            ones_r = const.tile([128, 128], F32R)
            nc.vector.tensor_copy(ones_r[:], ones_f[:])
            ident = const.tile([128, 128], F32)
            make_identity(nc, ident)

            # ---- resident activations ----
            k_rot = const.tile([128, S], F32R)          # K, (d, l) layout
            v_t = const.tile([128, S // 128, 128], F32R)  # V^T, (l % 128, l//128, d)
            # attn out, one tile per (g, si) so o_proj deps stay per-slice
            out_t = [[const.tile([128, ST], F32R, name=f"out_{g}_{si}")
                      for si in range(NST)] for g in range(G)]

            def rope(raw_r, dst_ap, sl):
                """dst = raw*cos + (rot@raw)*sin over s-slice sl."""
                ps_rot = ps.tile([128, ST], F32, tag="mm", bufs=4, name="ps_rot")
                nc.tensor.matmul(ps_rot[:], rot_sb[:], raw_r[:],
                                 start=True, stop=True)
                t1 = sb.tile([128, ST], F32, tag="rope_t1", bufs=2)
                t2 = sb.tile([128, ST], F32, tag="rope_t2", bufs=2)
                nc.vector.tensor_tensor(t1[:], raw_r[:], cos_sb[:, sl], Alu.mult)
                nc.vector.tensor_tensor(t2[:], ps_rot[:], sin_sb[:, sl], Alu.mult)
                nc.vector.tensor_tensor(dst_ap, t1[:], t2[:], Alu.add)

            for si in range(NST):
                sl = slice(si * ST, (si + 1) * ST)
                # ---------- QKV projections for this s-tile ----------
                ps_q = [ps.tile([128, ST], F32, tag="mm", bufs=4, name=f"ps_q{g}")
                        for g in range(G)]
                ps_kv = ps.tile([128, 2 * ST], F32, tag="big", bufs=2)
                ps_k = ps_kv[:, 0:ST]
                ps_v = ps_kv[:, ST:2 * ST]
                for ko in range(KO):
                    xt = sb.tile([128, ST], F32R, tag="x", bufs=8)
                    nc.sync.dma_start(xt[:], x[ko][:, sl])
                    wqt = sb.tile([128, G * 128], F32R, tag="wq", bufs=12)
                    nc.sync.dma_start(wqt[:], wq[ko])
                    st = (ko == 0)
                    sp = (ko == KO - 1)
                    for g in range(G):
                        nc.tensor.matmul(ps_q[g][:],
                                         wqt[:, g * 128:(g + 1) * 128], xt[:],
                                         start=st, stop=sp)
                    nc.tensor.matmul(ps_k, wk_sb[:, ko, :], xt[:],
                                     start=st, stop=sp)
                    nc.tensor.matmul(ps_v, wv_sb[:, ko, :], xt[:],
                                     start=st, stop=sp)

                # ---------- K: bias + rope into resident k_rot ----------
                k_raw = sb.tile([128, ST], F32R, tag="k_raw", bufs=2)
                nc.vector.tensor_scalar(k_raw[:], ps_k, bk_sb[:, 0:1], None,
                                        Alu.add)
                rope(k_raw, k_rot[:, sl], sl)

                # ---------- V: bias, then transpose into v_t ----------
                v_sb = sb.tile([128, ST], F32, tag="v_sb", bufs=2)
                nc.vector.tensor_scalar(v_sb[:], ps_v, bv_sb[:, 0:1], None,
                                        Alu.add)
                for j in range(ST // 128):
                    ps_t = ps.tile([128, 128], F32, tag="mm", bufs=4)
                    nc.tensor.transpose(ps_t[:], v_sb[:, j * 128:(j + 1) * 128],
                                        ident[:])
                    nc.vector.tensor_copy(v_t[:, si * (ST // 128) + j, :], ps_t[:])

                # ---------- rope all 4 query heads up front ----------
                nli = (si + 1) * (ST // 128)  # visible l-blocks
                q_rots = []
                for g in range(G):
                    q_raw = sb.tile([128, ST], F32R, tag="q_raw", bufs=2,
                                    name=f"q_raw{g}")
                    nc.vector.tensor_scalar(q_raw[:], ps_q[g][:],
                                            bq_sb[:, g:g + 1], INV_SQRT_D,
                                            Alu.add, Alu.mult)
                    q_rot = sb.tile([128, ST], F32R, tag="q_rot", bufs=4,
                                    name=f"q_rot{g}")
                    rope(q_raw, q_rot[:], sl)
                    q_rots.append(q_rot)

                # ---------- attention per query head ----------
                for g in range(G):
                    q_rot = q_rots[g]
                    ps_av = ps.tile([128, ST], F32, tag="mm", bufs=4)
                    ps_den = ps.tile([128, ST], F32, tag="mm", bufs=4)
                    for pi in range(nli // 2):
                        ps_s2 = ps.tile([128, 2 * ST], F32, tag="big", bufs=2)
                        offs = []
                        for h in range(2):
                            li = 2 * pi + h
                            j = li - si * (ST // 128)
                            # diagonal block j: columns [0, 128j) are fully
                            # masked -> skip them (affine_select zeroes the
                            # garbage left in psum/p there)
                            off = 128 * j if j > 0 else 0
                            offs.append(off)
                            nc.tensor.matmul(
                                ps_s2[:, h * ST + off:(h + 1) * ST],
                                k_rot[:, li * 128:(li + 1) * 128],
                                q_rot[:, off:], start=True, stop=True)
                        p2 = sb.tile([128, 2 * ST], F32R, tag="p", bufs=3)
                        nc.scalar.activation(p2[:], ps_s2[:], ActF.Exp)
                        for h in range(2):
                            li = 2 * pi + h
                            j = li - si * (ST // 128)
                            if j >= 0:
                                # causal: keep where l <= s (ds - dl - 128j >= 0)
                                nc.gpsimd.affine_select(
                                    out=p2[:, h * ST:(h + 1) * ST],
                                    in_=p2[:, h * ST:(h + 1) * ST],
                                    compare_op=Alu.is_ge, fill=0.0,
                                    base=-128 * j, channel_multiplier=-1,
                                    pattern=[[1, ST]],
                                )
                        for h in range(2):
                            li = 2 * pi + h
                            off = offs[h]
                            ph = p2[:, h * ST + off:(h + 1) * ST]
                            nc.tensor.matmul(ps_av[:, off:], v_t[:, li, :], ph,
                                             start=(li == 0),
                                             stop=(li == nli - 1))
                            nc.tensor.matmul(ps_den[:, off:], ones_r[:], ph,
                                             start=(li == 0),
                                             stop=(li == nli - 1))
                    # evict accumulators fast so their PSUM slots recycle;
                    # the slow reciprocal then runs off the critical path
                    den_sb = sb.tile([128, ST], F32, tag="den_sb", bufs=2)
                    nc.vector.tensor_copy(den_sb[:], ps_den[:])
                    av_sb = sb.tile([128, ST], F32, tag="av_sb", bufs=2)
                    nc.vector.tensor_copy(av_sb[:], ps_av[:])
                    recip = sb.tile([128, ST], F32, tag="recip", bufs=2)
                    nc.vector.reciprocal(recip[:], den_sb[:])
                    nc.vector.tensor_tensor(out_t[g][si][:], av_sb[:],
                                            recip[:], Alu.mult)

            # ---------- o_proj: y_partial = wo_colshard @ out ----------
            # chunked: after each group of 8 row-blocks, ReduceScatter that
            # chunk (overlaps the collective with the next group's compute)
            NCHUNK = G  # 4 chunks of 8 row-blocks
            MO_PER = MO // NCHUNK
            cc_in = dram.tile([MO, 128, S], F32)
            cc_out = dram.tile([NCHUNK, 128, S], F32)
            for chunk in range(NCHUNK):
                for mo in range(chunk * MO_PER, (chunk + 1) * MO_PER):
                    wot = sb.tile([128, G, 128], F32R, tag="wo", bufs=4)
                    # scalar-engine DMA queue: keeps these reads from queuing
                    # behind the y-tile writes on the sync queue
                    nc.scalar.dma_start(wot[:], wo[mo])
                    for si in range(NST):
                        ps_y = ps.tile([128, ST], F32, tag="mm", bufs=4)
                        for g in range(G):
                            nc.tensor.matmul(ps_y[:], wot[:, g, :],
                                             out_t[g][si][:],
                                             start=(g == 0), stop=(g == G - 1))
                        y_sb = sb.tile([128, ST], F32, tag="y_sb", bufs=4)
                        dst = cc_in[mo][:, si * ST:(si + 1) * ST]
                        # split evictions AND their writeback DMA queues so
                        # y-writes never back up a single queue
                        if (mo + si) % 2 == 0:
                            nc.scalar.activation(y_sb[:], ps_y[:], ActF.Copy)
                            nc.gpsimd.dma_start(dst, y_sb[:])
                        else:
                            nc.vector.tensor_copy(y_sb[:], ps_y[:])
                            nc.sync.dma_start(dst, y_sb[:])
                if with_collective:
                    # core c receives row-block mo = chunk*8 + c
                    nc.gpsimd.collective_compute(
                        "ReduceScatter",
                        Alu.add,
                        replica_groups=[list(range(NCORES))],
                        ins=[cc_in[chunk * MO_PER:(chunk + 1) * MO_PER].opt()],
                        outs=[cc_out[chunk:chunk + 1].opt()],
                    )
                    nc.sync.dma_start(yout[chunk:chunk + 1], cc_out[chunk:chunk + 1])
            if not with_collective:
                # profiling-only variant: local copy instead of the
                # collective (output is the unreduced local shard)
                nc.sync.dma_start(yout[:, :, :], cc_in[MO - G:MO])

    nc.compile()
    return nc


def _rot_matrix():
    # q_rot = R @ q with rotate_half along D: R @ v = concat(-v[64:], v[:64])
    R = np.zeros((128, 128), np.float32)
    for i in range(64):
        R[i, 64 + i] = -1.0
        R[64 + i, i] = 1.0
    return R


def _prep_in_maps(inputs):
    x = np.ascontiguousarray(np.asarray(inputs["hidden_states"],
                                        np.float32)[0, :, 0, :])
    wq = np.asarray(inputs["wq"], np.float32)
    wk = np.asarray(inputs["wk"], np.float32)
    wv = np.asarray(inputs["wv"], np.float32)
    wo = np.asarray(inputs["wo"], np.float32)
    bq = np.asarray(inputs["bq"], np.float32)
    bk = np.asarray(inputs["bk"], np.float32)
    bv = np.asarray(inputs["bv"], np.float32)
    cos_t = np.ascontiguousarray(np.asarray(inputs["cos_t"],
                                            np.float32)[0, 0])  # (128, S)
    sin_t = np.ascontiguousarray(np.asarray(inputs["sin_t"], np.float32)[0, 0])
    rotT = np.ascontiguousarray(_rot_matrix().T)

    x_r = np.ascontiguousarray(x.reshape(KO, 128, S))
    in_maps = []
    for c in range(NCORES):
        qs = slice(c * G * 128, (c + 1) * G * 128)
        ks = slice(c * 128, (c + 1) * 128)
        wq_t = np.ascontiguousarray(wq[qs].T.reshape(KO, 128, G * 128))
        wk_t = np.ascontiguousarray(
            wk[ks].T.reshape(KO, 128, 128).transpose(1, 0, 2))
        wv_t = np.ascontiguousarray(
            wv[ks].T.reshape(KO, 128, 128).transpose(1, 0, 2))
        # wo column shard -> (mo, d, g, m): woT[g*128+d, mo*128+m]
        wo_t = np.ascontiguousarray(
            wo[:, qs].T.reshape(G, 128, MO, 128).transpose(2, 1, 0, 3))
        in_maps.append({
            "x": x_r,
            "wq": wq_t,
            "wk": wk_t,
            "wv": wv_t,
            "wo": wo_t,
            "bq": np.ascontiguousarray(bq[qs].reshape(G, 128).T),
            "bk": np.ascontiguousarray(bk[ks][:, None]),
            "bv": np.ascontiguousarray(bv[ks][:, None]),
            "cos": cos_t,
            "sin": sin_t,
            "rot": rotT,
        })
    return in_maps


_NC = None


def _get_nc():
    global _NC
    if _NC is None:
        _NC = build_nc()
    return _NC


def assemble_output(results):
    """Chunked ReduceScatter: core c's chunk i is y row-block mo = 8*i + c."""
    y = np.empty((HID, S), np.float32)
    for c in range(NCORES):
        yc = results[c]["y"]
        for i in range(yc.shape[0]):
            mo = NCORES * i + c
            y[mo * 128:(mo + 1) * 128] = yc[i]
    return y[None, :, None, :]


def kernel(**inputs):
    nc = _get_nc()
    in_maps = _prep_in_maps(inputs)
    res = run_bass_kernel_spmd(nc, in_maps, core_ids=list(range(NCORES)))
    return assemble_output(res.results)



# revision 4
# speedup vs baseline: 1.2828x; 1.2828x over previous
"""Trainium2 Bass kernel for AttentionPatcher (GQA attention block, S=2048).

Sharding: 8-way tensor parallel over KV head groups. Core c owns KV head c
and query heads 4c..4c+3: it computes its Q/K/V projections, RoPE, causal
attention, and a full partial o_proj (wo column shard); a ReduceScatter(add)
over the 8 cores then leaves core c with rows [512c, 512c+512) of the final
output, which the host concatenates.

All matmul operands are bf16 (PSUM accumulation stays fp32): bf16 halves
LDWEIGHTS time so the PE streams at ~1 row/cycle, and halves all DMA
traffic. wq stays resident in SBUF (loaded once, not re-streamed per
s-tile). The softmax denominator is accumulated across l-blocks on the
vector engine (one ones-matmul per head-tile instead of one per l-block),
and exp/masking only touch the causally-visible region.
"""
import os
import sys

import numpy as np
import ml_dtypes

if os.path.isdir("/opt/trn_rl_repo") and "/opt/trn_rl_repo" not in sys.path:
    sys.path.insert(0, "/opt/trn_rl_repo")

import concourse.bacc as bacc
import concourse.mybir as mybir
import concourse.tile as tile
from concourse.bass_utils import run_bass_kernel_spmd
from concourse.masks import make_identity

F32 = mybir.dt.float32
BF16 = mybir.dt.bfloat16
ActF = mybir.ActivationFunctionType
Alu = mybir.AluOpType
NBF = ml_dtypes.bfloat16

H, KV, D, S = 32, 8, 128, 2048
HID = H * D
NCORES = 8
G = H // KV          # query heads per core
ST = 512             # s-tile size
NST = S // ST        # 4 s-tiles
KO = HID // 128      # 32 contraction subtiles
MO = HID // 128      # 32 output row tiles
INV_SQRT_D = 1.0 / float(np.sqrt(D))


def build_nc(with_collective=True):
    nc = bacc.Bacc("TRN2", target_bir_lowering=False, debug=False)

    x = nc.dram_tensor("x", [KO, 128, S], BF16, kind="ExternalInput")
    wq = nc.dram_tensor("wq", [128, KO, G * 128], BF16, kind="ExternalInput")
    wk = nc.dram_tensor("wk", [128, KO, 128], BF16, kind="ExternalInput")
    wv = nc.dram_tensor("wv", [128, KO, 128], BF16, kind="ExternalInput")
    wo = nc.dram_tensor("wo", [MO, 128, G, 128], BF16, kind="ExternalInput")
    bq = nc.dram_tensor("bq", [128, G], F32, kind="ExternalInput")
    bk = nc.dram_tensor("bk", [128, 1], F32, kind="ExternalInput")
    bv = nc.dram_tensor("bv", [128, 1], F32, kind="ExternalInput")
    cos = nc.dram_tensor("cos", [128, S], BF16, kind="ExternalInput")
    sin = nc.dram_tensor("sin", [128, S], BF16, kind="ExternalInput")
    rot = nc.dram_tensor("rot", [128, 128], BF16, kind="ExternalInput")
    yout = nc.dram_tensor("y", [G, 128, S], BF16, kind="ExternalOutput")

    with tile.TileContext(nc) as tc:
        with (
            tc.tile_pool(name="const", bufs=1) as const,
            tc.tile_pool(name="sb", bufs=3) as sb,
            tc.tile_pool(name="ps", bufs=1, space="PSUM") as ps,
            tc.tile_pool(name="dram", bufs=1, space="DRAM") as dram,
        ):
            # ---- resident constants ----
            # chunked across the scalar/gpsimd/vector DMA queues so the sync
            # queue starts streaming x immediately and the first ko chunks
            # land fast
            wq_sb = const.tile([128, KO, G * 128], BF16)
            wk_sb = const.tile([128, KO, 128], BF16)
            wv_sb = const.tile([128, KO, 128], BF16)
            for c8 in range(8):
                ksl = slice(c8 * (KO // 8), (c8 + 1) * (KO // 8))
                nc.scalar.dma_start(wq_sb[:, ksl, :], wq[:, ksl, :])
                nc.gpsimd.dma_start(wk_sb[:, ksl, :], wk[:, ksl, :])
                nc.gpsimd.dma_start(wv_sb[:, ksl, :], wv[:, ksl, :])
            bq_sb = const.tile([128, G], F32)
            bk_sb = const.tile([128, 1], F32)
            bv_sb = const.tile([128, 1], F32)
            nc.gpsimd.dma_start(bq_sb[:], bq[:, :])
            nc.gpsimd.dma_start(bk_sb[:], bk[:, :])
            nc.gpsimd.dma_start(bv_sb[:], bv[:, :])
            rot_sb = const.tile([128, 128], BF16)
            nc.gpsimd.dma_start(rot_sb[:], rot[:, :])
            cos_sb = const.tile([128, S], BF16)
            sin_sb = const.tile([128, S], BF16)
            nc.gpsimd.dma_start(cos_sb[:], cos[:, :])
            nc.gpsimd.dma_start(sin_sb[:], sin[:, :])
            ones_f = const.tile([128, 128], F32)
            nc.any.memset(ones_f[:], 1.0)
            ones_b = const.tile([128, 128], BF16)
            nc.vector.tensor_copy(ones_b[:], ones_f[:])
            ident = const.tile([128, 128], F32)
            make_identity(nc, ident)

            # ---- resident activations ----
            k_rot = const.tile([128, S], BF16)          # K, (d, l) layout
            v_t = const.tile([128, S // 128, 128], BF16)  # V^T (l%128, l//128, d)
            # attn out, one tile per (g, si) so o_proj deps stay per-slice
            out_t = [[const.tile([128, ST], BF16, name=f"out_{g}_{si}")
                      for si in range(NST)] for g in range(G)]

            def rope(raw_b, dst_ap, sl):
                """dst = raw*cos + (rot@raw)*sin over s-slice sl (bf16)."""
                ps_rot = ps.tile([128, ST], F32, tag="mm", bufs=4, name="ps_rot")
                nc.tensor.matmul(ps_rot[:], rot_sb[:], raw_b[:],
                                 start=True, stop=True)
                t1 = sb.tile([128, ST], BF16, tag="rope_t1", bufs=2)
                t2 = sb.tile([128, ST], BF16, tag="rope_t2", bufs=2)
                nc.vector.tensor_tensor(t1[:], raw_b[:], cos_sb[:, sl], Alu.mult)
                nc.vector.tensor_tensor(t2[:], ps_rot[:], sin_sb[:, sl], Alu.mult)
                nc.vector.tensor_tensor(dst_ap, t1[:], t2[:], Alu.add)

            for si in range(NST):
                sl = slice(si * ST, (si + 1) * ST)
                # ---------- QKV projections for this s-tile ----------
                ps_q = [ps.tile([128, ST], F32, tag="mm", bufs=4, name=f"ps_q{g}")
                        for g in range(G)]
                ps_kv = ps.tile([128, 2 * ST], F32, tag="big", bufs=2)
                ps_k = ps_kv[:, 0:ST]
                ps_v = ps_kv[:, ST:2 * ST]
                for ko in range(KO):
                    xt = sb.tile([128, ST], BF16, tag="x", bufs=12)
                    nc.sync.dma_start(xt[:], x[ko][:, sl])
                    st = (ko == 0)
                    sp = (ko == KO - 1)
                    for g in range(G):
                        nc.tensor.matmul(ps_q[g][:],
                                         wq_sb[:, ko, g * 128:(g + 1) * 128],
                                         xt[:], start=st, stop=sp)
                    nc.tensor.matmul(ps_k, wk_sb[:, ko, :], xt[:],
                                     start=st, stop=sp)
                    nc.tensor.matmul(ps_v, wv_sb[:, ko, :], xt[:],
                                     start=st, stop=sp)

                # ---------- K: bias + rope into resident k_rot ----------
                k_raw = sb.tile([128, ST], BF16, tag="k_raw", bufs=2)
                nc.vector.tensor_scalar(k_raw[:], ps_k, bk_sb[:, 0:1], None,
                                        Alu.add)
                rope(k_raw, k_rot[:, sl], sl)

                # ---------- V: bias, then transpose into v_t ----------
                v_sb = sb.tile([128, ST], F32, tag="v_sb", bufs=2)
                nc.vector.tensor_scalar(v_sb[:], ps_v, bv_sb[:, 0:1], None,
                                        Alu.add)
                for j in range(ST // 128):
                    ps_t = ps.tile([128, 128], F32, tag="mm", bufs=4)
                    nc.tensor.transpose(ps_t[:], v_sb[:, j * 128:(j + 1) * 128],
                                        ident[:])
                    nc.vector.tensor_copy(v_t[:, si * (ST // 128) + j, :], ps_t[:])

                # ---------- rope all 4 query heads up front ----------
                nli = (si + 1) * (ST // 128)  # visible l-blocks
                q_rots = []
                for g in range(G):
                    q_raw = sb.tile([128, ST], BF16, tag="q_raw", bufs=2,
                                    name=f"q_raw{g}")
                    nc.vector.tensor_scalar(q_raw[:], ps_q[g][:],
                                            bq_sb[:, g:g + 1], INV_SQRT_D,
                                            Alu.add, Alu.mult)
                    q_rot = sb.tile([128, ST], BF16, tag="q_rot", bufs=4,
                                    name=f"q_rot{g}")
                    rope(q_raw, q_rot[:], sl)
                    q_rots.append(q_rot)

                # ---------- attention per query head ----------
                for g in range(G):
                    q_rot = q_rots[g]
                    ps_av = ps.tile([128, ST], F32, tag="mm", bufs=4)
                    den_acc = sb.tile([128, ST], BF16, tag="den_acc", bufs=2)
                    for pi in range(nli // 2):
                        ps_s2 = ps.tile([128, 2 * ST], F32, tag="big", bufs=2)
                        offs = []
                        for h in range(2):
                            li = 2 * pi + h
                            j = li - si * (ST // 128)
                            # diagonal block j: columns [0, 128j) are fully
                            # masked -> skip them entirely
                            off = 128 * j if j > 0 else 0
                            offs.append(off)
                            nc.tensor.matmul(
                                ps_s2[:, h * ST + off:(h + 1) * ST],
                                k_rot[:, li * 128:(li + 1) * 128],
                                q_rot[:, off:], start=True, stop=True)
                        p2 = sb.tile([128, 2 * ST], BF16, tag="p", bufs=3)
                        if offs[0] == 0 and offs[1] == 0:
                            nc.scalar.activation(p2[:], ps_s2[:], ActF.Exp)
                        else:
                            for h in range(2):
                                off = offs[h]
                                nc.scalar.activation(
                                    p2[:, h * ST + off:(h + 1) * ST],
                                    ps_s2[:, h * ST + off:(h + 1) * ST],
                                    ActF.Exp)
                        for h in range(2):
                            li = 2 * pi + h
                            j = li - si * (ST // 128)
                            if j >= 0:
                                # causal: triangular mask only touches the
                                # 128-col diagonal sub-block (keep s >= l)
                                off = offs[h]
                                nc.gpsimd.affine_select(
                                    out=p2[:, h * ST + off:h * ST + off + 128],
                                    in_=p2[:, h * ST + off:h * ST + off + 128],
                                    compare_op=Alu.is_ge, fill=0.0,
                                    base=0, channel_multiplier=-1,
                                    pattern=[[1, 128]],
                                )
                        for h in range(2):
                            li = 2 * pi + h
                            off = offs[h]
                            ph = p2[:, h * ST + off:(h + 1) * ST]
                            nc.tensor.matmul(ps_av[:, off:], v_t[:, li, :], ph,
                                             start=(li == 0),
                                             stop=(li == nli - 1))
                            # softmax denominator: accumulate exp(p) blocks on
                            # the vector engine (summed over partitions by a
                            # single ones-matmul per (g, si) below)
                            if li == 0:
                                nc.vector.tensor_copy(den_acc[:], ph)
                            else:
                                nc.vector.tensor_tensor(
                                    den_acc[:, off:], den_acc[:, off:], ph,
                                    Alu.add)
                    ps_den = ps.tile([128, ST], F32, tag="mm", bufs=4)
                    nc.tensor.matmul(ps_den[:], ones_b[:], den_acc[:],
                                     start=True, stop=True)
                    # evict accumulators fast so their PSUM slots recycle;
                    # the slow reciprocal then runs off the critical path
                    den_sb = sb.tile([128, ST], F32, tag="den_sb", bufs=2)
                    nc.vector.tensor_copy(den_sb[:], ps_den[:])
                    av_sb = sb.tile([128, ST], F32, tag="av_sb", bufs=2)
                    nc.vector.tensor_copy(av_sb[:], ps_av[:])
                    recip = sb.tile([128, ST], F32, tag="recip", bufs=2)
                    nc.vector.reciprocal(recip[:], den_sb[:])
                    nc.vector.tensor_tensor(out_t[g][si][:], av_sb[:],
                                            recip[:], Alu.mult)

            # ---------- o_proj: y_partial = wo_colshard @ out ----------
            # chunked: after each group of 8 row-blocks, ReduceScatter that
            # chunk (overlaps the collective with the next group's compute)
            NCHUNK = G  # 4 chunks of 8 row-blocks
            MO_PER = MO // NCHUNK
            cc_in = dram.tile([MO, 128, S], BF16)
            cc_out = dram.tile([NCHUNK, 128, S], BF16)
            for chunk in range(NCHUNK):
                for mo in range(chunk * MO_PER, (chunk + 1) * MO_PER):
                    wot = sb.tile([128, G, 128], BF16, tag="wo", bufs=6)
                    # scalar-engine DMA queue: keeps these reads from queuing
                    # behind the y-tile writes on the sync queue
                    nc.scalar.dma_start(wot[:], wo[mo])
                    for si in range(NST):
                        ps_y = ps.tile([128, ST], F32, tag="mm", bufs=4)
                        for g in range(G):
                            nc.tensor.matmul(ps_y[:], wot[:, g, :],
                                             out_t[g][si][:],
                                             start=(g == 0), stop=(g == G - 1))
                        y_sb = sb.tile([128, ST], BF16, tag="y_sb", bufs=4)
                        dst = cc_in[mo][:, si * ST:(si + 1) * ST]
                        # split evictions AND their writeback DMA queues so
                        # y-writes never back up a single queue
                        if (mo + si) % 2 == 0:
                            nc.scalar.activation(y_sb[:], ps_y[:], ActF.Copy)
                            nc.gpsimd.dma_start(dst, y_sb[:])
                        else:
                            nc.vector.tensor_copy(y_sb[:], ps_y[:])
                            nc.sync.dma_start(dst, y_sb[:])
                if with_collective:
                    # core c receives row-block mo = chunk*8 + c
                    nc.gpsimd.collective_compute(
                        "ReduceScatter",
                        Alu.add,
                        replica_groups=[list(range(NCORES))],
                        ins=[cc_in[chunk * MO_PER:(chunk + 1) * MO_PER].opt()],
                        outs=[cc_out[chunk:chunk + 1].opt()],
                    )
                    nc.sync.dma_start(yout[chunk:chunk + 1],
                                      cc_out[chunk:chunk + 1])
                else:
                    # profiling-only variant: local per-chunk copy instead of
                    # the collective (same yout DMA shape; output is an
                    # unreduced local shard)
                    src = chunk * MO_PER + chunk
                    nc.sync.dma_start(yout[chunk:chunk + 1],
                                      cc_in[src:src + 1])

    nc.compile()
    return nc


def _rot_matrix():
    # q_rot = R @ q with rotate_half along D: R @ v = concat(-v[64:], v[:64])
    R = np.zeros((128, 128), np.float32)
    for i in range(64):
        R[i, 64 + i] = -1.0
        R[64 + i, i] = 1.0
    return R


def _prep_in_maps(inputs):
    x = np.ascontiguousarray(np.asarray(inputs["hidden_states"],
                                        np.float32)[0, :, 0, :])
    wq = np.asarray(inputs["wq"], np.float32)
    wk = np.asarray(inputs["wk"], np.float32)
    wv = np.asarray(inputs["wv"], np.float32)
    wo = np.asarray(inputs["wo"], np.float32)
    bq = np.asarray(inputs["bq"], np.float32)
    bk = np.asarray(inputs["bk"], np.float32)
    bv = np.asarray(inputs["bv"], np.float32)
    cos_t = np.ascontiguousarray(
        np.asarray(inputs["cos_t"], np.float32)[0, 0]).astype(NBF)  # (128, S)
    sin_t = np.ascontiguousarray(
        np.asarray(inputs["sin_t"], np.float32)[0, 0]).astype(NBF)
    rotT = np.ascontiguousarray(_rot_matrix().T).astype(NBF)

    x_r = np.ascontiguousarray(x.reshape(KO, 128, S)).astype(NBF)
    in_maps = []
    for c in range(NCORES):
        qs = slice(c * G * 128, (c + 1) * G * 128)
        ks = slice(c * 128, (c + 1) * 128)
        wq_t = np.ascontiguousarray(
            wq[qs].T.reshape(KO, 128, G * 128).transpose(1, 0, 2)).astype(NBF)
        wk_t = np.ascontiguousarray(
            wk[ks].T.reshape(KO, 128, 128).transpose(1, 0, 2)).astype(NBF)
        wv_t = np.ascontiguousarray(
            wv[ks].T.reshape(KO, 128, 128).transpose(1, 0, 2)).astype(NBF)
        # wo column shard -> (mo, d, g, m): woT[g*128+d, mo*128+m]
        wo_t = np.ascontiguousarray(
            wo[:, qs].T.reshape(G, 128, MO, 128).transpose(2, 1, 0, 3)
        ).astype(NBF)
        in_maps.append({
            "x": x_r,
            "wq": wq_t,
            "wk": wk_t,
            "wv": wv_t,
            "wo": wo_t,
            "bq": np.ascontiguousarray(bq[qs].reshape(G, 128).T),
            "bk": np.ascontiguousarray(bk[ks][:, None]),
            "bv": np.ascontiguousarray(bv[ks][:, None]),
            "cos": cos_t,
            "sin": sin_t,
            "rot": rotT,
        })
    return in_maps


_NC = None


def _get_nc():
    global _NC
    if _NC is None:
        _NC = build_nc()
    return _NC


def assemble_output(results):
    """Chunked ReduceScatter: core c's chunk i is y row-block mo = 8*i + c."""
    y = np.empty((HID, S), np.float32)
    for c in range(NCORES):
        yc = results[c]["y"]
        for i in range(yc.shape[0]):
            mo = NCORES * i + c
            y[mo * 128:(mo + 1) * 128] = yc[i].astype(np.float32)
    return y[None, :, None, :]


def kernel(**inputs):
    nc = _get_nc()
    in_maps = _prep_in_maps(inputs)
    res = run_bass_kernel_spmd(nc, in_maps, core_ids=list(range(NCORES)))
    return assemble_output(res.results)
